# revision 9
# baseline (speedup 1.0000x reference)
"""Trainium2 Bass kernel: per-pixel channel shuffle + 3x3 conv (stride 1, pad 1).

Problem: x [32,256,56,56] f32, w [256,256,3,3] f32 (OIHW), perm [3136,256] i32;
out[b,:,h,w] = conv3x3(xs)[b,:,h,w] where xs[b,:,l] = x[b, perm[l,:], l].

Strategy (8 NeuronCores, data-parallel over batch, 4 batches/core):
  Shuffle (per batch-pair, per 112-pixel tile): host pre-transposes x to
  [l, c] layout, DMA tiles straight into SBUF, one GPSIMD local_scatter per
  tile applies per-pixel inverse channel perms for BOTH batches of the pair
  (512 idxs), PE-transpose back -> [c, l] into a zero-padded 58x58 image xs
  stored row-interleaved ([row][ct][col]) so row-band consumers get precise
  sub-tile deps.
  Conv: 1-D Winograd F(2,3) along x (1.5x fewer PE MACs than direct). V[j]
  computed per 14-row group (fine-grained pipeline with the shuffle); M[j]
  accumulates 3 kh-taps x 2 ic-tiles in PSUM; output transform Y = A^T-combos
  of M[j] on DVE/Scalar, writing interleaved output column pairs.
"""

import os
import sys
import types
import numpy as np

_STATE = {}
LAST_RESULT = None

B, C, H, W = 32, 256, 56, 56
HW = H * W
PADW = 58
XROW = 2 * PADW        # one padded row holds both ct halves
XS_TOT = 58 * XROW
TL = 112
NT = 28
N_CORES = 8
B_LOC = B // N_CORES
TX = 28                # winograd column tiles (pairs of output columns)
NG = 392               # matmul N: 14 output rows x 28 tiles
NGRP = 4               # row groups of 14
VCH = 16 * TX          # V chunk per ct: 16 rows x 28 tiles


def _tid1(j, kh, ct, oc):
    return ((j * 3 + kh) * 2 + ct) * 2 + oc


def _install_ntff_shim():
    name = "antenv.axon_hooks"
    if name in sys.modules:
        return
    try:
        import antenv  # noqa: F401

        m = types.ModuleType(name)
        m._hook = None
        m.set_axon_ntff_profile_hook = lambda h: setattr(m, "_hook", h)
        m.get_axon_ntff_profile_hook = lambda: m._hook
        sys.modules[name] = m
        setattr(sys.modules["antenv"], "axon_hooks", m)
        from trn_agent_boot.trn_boot import _ntff_profile_via_ctypes

        hook = _ntff_profile_via_ctypes("/opt/axon/libaxon_pjrt.so")
        if hook is not None:
            m.set_axon_ntff_profile_hook(hook)
    except Exception:
        pass


def _build_kernel():
    import concourse.bass as bass
    import concourse.mybir as mybir
    from concourse import bacc, tile
    from concourse.masks import make_identity
    from contextlib import ExitStack

    F32 = mybir.dt.float32
    BF16 = mybir.dt.bfloat16
    I16 = mybir.dt.int16

    nc = bacc.Bacc("TRN2", target_bir_lowering=False, debug=False, num_devices=N_CORES)

    xbt = nc.dram_tensor("xbt", [B_LOC, HW, C], BF16, kind="ExternalInput")
    u1 = nc.dram_tensor("u1", [48, 128, 128], BF16, kind="ExternalInput")
    idxt = nc.dram_tensor("idxt", [128, NT * 512], I16, kind="ExternalInput")
    out = nc.dram_tensor("out", [B_LOC, C, HW], F32, kind="ExternalOutput")

    with tile.TileContext(nc) as tc, ExitStack() as ctx:
        const = ctx.enter_context(tc.tile_pool(name="const", bufs=1))
        usb = const.tile([128, 48 * 128], BF16)
        nc.sync.dma_start(
            out=usb[:, :],
            in_=bass.AP(u1, 0, [[128, 128], [128 * 128, 48], [1, 128]]),
        )
        idxsb = const.tile([128, NT * 512], I16)
        nc.sync.dma_start(out=idxsb[:, :], in_=idxt[:, :])
        ident = const.tile([128, 128], BF16)
        make_identity(nc, ident[:, :])

        xs_pool = ctx.enter_context(tc.tile_pool(name="xs", bufs=4))
        gin_pool = ctx.enter_context(tc.tile_pool(name="gin", bufs=5))
        sout_pool = ctx.enter_context(tc.tile_pool(name="sout", bufs=5))
        v_pool = ctx.enter_context(tc.tile_pool(name="vp", bufs=12))
        tt_pool = ctx.enter_context(tc.tile_pool(name="tt", bufs=4))
        y_pool = ctx.enter_context(tc.tile_pool(name="yb", bufs=4))
        tpsB_pool = ctx.enter_context(tc.tile_pool(name="tpsB", bufs=3, space="PSUM"))
        z_pool = ctx.enter_context(tc.tile_pool(name="zp", bufs=4, space="PSUM"))

        def phase2(b, xs):
            # d_b view for row group g: [p, yy, ct, tx, 1] =
            #   xs[p, (g*14+yy)*116 + ct*58 + 2*tx + b_]
            def dview(b_, g):
                v = xs[:, :].rearrange("p (yy u) -> p yy u", u=XROW)
                v = v[:, g * 14 : g * 14 + 16, :]
                v = v.rearrange("p yy (ct u2) -> p yy ct u2", ct=2)
                v = v.rearrange("p yy ct (vv ww) -> p yy ct vv ww", ww=2)
                return v[:, :, :, b_ // 2 : b_ // 2 + TX, b_ % 2 : b_ % 2 + 1]

            for g in range(NGRP):
                vt = []
                for j in range(4):
                    vj = v_pool.tile([128, 2 * VCH], BF16, name="vj")
                    vv = vj[:, :].rearrange(
                        "p (ct yy tx q) -> p yy ct tx q", ct=2, tx=TX, q=1
                    )
                    if j == 0:
                        nc.vector.tensor_sub(vv, dview(0, g), dview(2, g))
                    elif j == 1:
                        nc.vector.tensor_add(vv, dview(1, g), dview(2, g))
                    elif j == 2:
                        nc.vector.tensor_sub(vv, dview(2, g), dview(1, g))
                    else:
                        nc.vector.tensor_sub(vv, dview(1, g), dview(3, g))
                    vt.append(vj)

                for oc in range(2):
                    ybuf = y_pool.tile([128, 14 * 56], F32, name="ybuf")
                    yv = ybuf[:, :].rearrange("p (y c q) -> p y c q", y=14, q=2)
                    zt = {}
                    for stage, js in enumerate(((1, 2), (0, 3))):
                        for j in js:
                            z = z_pool.tile([128, NG], F32, name="z")
                            k = 0
                            for kh in range(3):
                                for ct in range(2):
                                    nc.tensor.matmul(
                                        z[:, :],
                                        lhsT=usb[
                                            :,
                                            _tid1(j, kh, ct, oc) * 128 : (
                                                _tid1(j, kh, ct, oc) + 1
                                            )
                                            * 128,
                                        ],
                                        rhs=vt[j][
                                            :,
                                            ct * VCH
                                            + kh * TX : ct * VCH
                                            + kh * TX
                                            + NG,
                                        ],
                                        start=(k == 0),
                                        stop=(k == 5),
                                    )
                                    k += 1
                            zt[j] = z
                        if stage == 0:
                            zc = tt_pool.tile([128, NG], F32, name="zc")
                            nc.scalar.copy(zc[:, :], zt[1][:, :])
                            t0 = tt_pool.tile([128, NG], F32, name="t0")
                            t1 = tt_pool.tile([128, NG], F32, name="t0")
                            nc.vector.tensor_add(t0[:, :], zc[:, :], zt[2][:, :])
                            nc.vector.tensor_sub(t1[:, :], zc[:, :], zt[2][:, :])
                    tshape = lambda ap: ap.rearrange(
                        "p (y c q) -> p y c q", y=14, c=28, q=1
                    )
                    nc.vector.tensor_add(
                        yv[:, :, :, 0:1], tshape(t0[:, :]), tshape(zt[0][:, :])
                    )
                    nc.vector.tensor_sub(
                        yv[:, :, :, 1:2], tshape(t1[:, :]), tshape(zt[3][:, :])
                    )
                    nc.sync.dma_start(
                        out=out[
                            b,
                            oc * 128 : (oc + 1) * 128,
                            g * 14 * 56 : (g + 1) * 14 * 56,
                        ],
                        in_=ybuf[:, :],
                    )

        for pair in range(B_LOC // 2):
            b0, b1 = 2 * pair, 2 * pair + 1
            xst = {}
            for b in (b0, b1):
                xs = xs_pool.tile([128, XS_TOT], BF16)
                xst[b] = xs
                xv = xs[:, :].rearrange("p (yy u) -> p yy u", u=XROW)
                nc.vector.memset(xs[:, 0:XROW], 0.0)
                nc.vector.memset(xs[:, 57 * XROW : XS_TOT], 0.0)
                for ct in range(2):
                    nc.vector.memset(xv[:, 1:57, ct * PADW : ct * PADW + 1], 0.0)
                    nc.vector.memset(
                        xv[:, 1:57, ct * PADW + 57 : ct * PADW + 58], 0.0
                    )

            for t in range(NT):
                gin = gin_pool.tile([128, 512], BF16, name="gin", tag="gin")
                for i, b in enumerate((b0, b1)):
                    nc.sync.dma_start(
                        out=gin[0:TL, i * 256 : (i + 1) * 256],
                        in_=xbt[b, t * TL : (t + 1) * TL, :],
                    )
                sout = sout_pool.tile([128, 512], BF16, name="sout", tag="sout")
                nc.gpsimd.local_scatter(
                    out_ap=sout[0:TL, :],
                    data_ap=gin[0:TL, :],
                    idxs_ap=idxsb[0:TL, t * 512 : (t + 1) * 512],
                    channels=TL,
                    num_elems=512,
                    num_idxs=512,
                )
                for i, b in enumerate((b0, b1)):
                    ps2 = tpsB_pool.tile([128, 2 * TL], BF16, name="ps2", tag="ps2")
                    for ct in range(2):
                        nc.tensor.transpose(
                            ps2[:, ct * TL : ct * TL + TL],
                            sout[0:TL, i * 256 + ct * 128 : i * 256 + ct * 128 + 128],
                            ident[0:TL, 0:TL],
                        )
                    for ct in range(2):
                        q = (2 * t + 1) * XROW + ct * PADW + 1
                        dst = xst[b][:, q : q + 2 * XROW].rearrange(
                            "p (r x) -> p r x", r=2
                        )[:, :, 0:56]
                        src = ps2[:, ct * TL : ct * TL + TL].rearrange(
                            "p (r x) -> p r x", r=2
                        )
                        if ct == 0:
                            nc.scalar.copy(dst, src)
                        else:
                            nc.vector.tensor_copy(dst, src)

            for b in (b0, b1):
                phase2(b, xst[b])

    nc.compile()
    return nc


def _host_prep(x, w, perm):
    import ml_dtypes

    # [B, HW, C] pixel-major bf16 (feeds the scatter without PE fwd transposes)
    xft = np.ascontiguousarray(
        x.reshape(B, C, HW).transpose(0, 2, 1)
    ).astype(ml_dtypes.bfloat16)

    # 1-D winograd weights: U1[j,kh][oc,ic] = sum_kw G[j,kw] w[oc,ic,kh,kw]
    G = np.array([[1, 0, 0], [0.5, 0.5, 0.5], [0.5, -0.5, 0.5], [0, 0, 1]], np.float32)
    U1 = np.einsum("jk,ochk->jhoc", G, w.astype(np.float32))  # [4,3,OC,C]
    u1t = np.empty((48, 128, 128), dtype=ml_dtypes.bfloat16)
    for j in range(4):
        for kh in range(3):
            for ct in range(2):
                for oc in range(2):
                    blk = U1[j, kh][oc * 128 : (oc + 1) * 128, ct * 128 : (ct + 1) * 128]
                    u1t[_tid1(j, kh, ct, oc)] = blk.T.astype(ml_dtypes.bfloat16)

    iperm = np.empty((HW, C), dtype=np.int16)
    np.put_along_axis(
        iperm, perm.astype(np.int64), np.arange(C, dtype=np.int16)[None, :], axis=1
    )
    idxt = np.zeros((128, NT * 512), dtype=np.int16)
    for t in range(NT):
        blk = iperm[t * TL : t * TL + TL, :]
        idxt[0:TL, t * 512 : t * 512 + 256] = blk
        idxt[0:TL, t * 512 + 256 : (t + 1) * 512] = blk + 256

    in_maps = []
    for cidx in range(N_CORES):
        in_maps.append(
            {
                "xbt": np.ascontiguousarray(xft[cidx * B_LOC : (cidx + 1) * B_LOC]),
                "u1": u1t,
                "idxt": idxt,
            }
        )
    return in_maps


def kernel(x, w, perm):
    global LAST_RESULT
    _install_ntff_shim()
    from concourse.bass_utils import run_bass_kernel_spmd

    x = np.asarray(x, dtype=np.float32)
    w = np.asarray(w, dtype=np.float32)
    perm = np.asarray(perm)

    if "nc" not in _STATE:
        _STATE["nc"] = _build_kernel()
    nc = _STATE["nc"]

    in_maps = _host_prep(x, w, perm)
    res = run_bass_kernel_spmd(nc, in_maps, core_ids=list(range(N_CORES)))
    LAST_RESULT = res
    out = np.concatenate(
        [r["out"].reshape(B_LOC, C, H, W) for r in res.results], axis=0
    )
    return out.astype(np.float32)


# revision 10
# speedup vs baseline: 1.3538x; 1.3538x over previous
"""Trainium2 Bass kernel: per-pixel channel shuffle + 3x3 conv (stride 1, pad 1).

Problem: x [32,256,56,56] f32, w [256,256,3,3] f32 (OIHW), perm [3136,256] i32;
out[b,:,h,w] = conv3x3(xs)[b,:,h,w] where xs[b,:,l] = x[b, perm[l,:], l].

Strategy (8 NeuronCores, data-parallel over batch, 4 batches/core):
  Shuffle (per batch-pair, per 112-pixel tile): host pre-transposes x to
  [l, c] layout, DMA tiles straight into SBUF, one GPSIMD local_scatter per
  tile applies per-pixel inverse channel perms for BOTH batches of the pair
  (512 idxs), PE-transpose back -> [c, l] into a zero-padded 58x58 image xs
  stored row-interleaved ([row][ct][col]) so row-band consumers get precise
  sub-tile deps.
  Conv: 1-D Winograd F(2,3) along x (1.5x fewer PE MACs than direct). V[j]
  computed per 14-row group (fine-grained pipeline with the shuffle); M[j]
  accumulates 3 kh-taps x 2 ic-tiles in PSUM; output transform Y = A^T-combos
  of M[j] on DVE/Scalar, writing interleaved output column pairs.
"""

import os
import sys
import types
import numpy as np

_STATE = {}
LAST_RESULT = None

B, C, H, W = 32, 256, 56, 56
HW = H * W
PADW = 58
XROW = 2 * PADW        # one padded row holds both ct halves
XS_TOT = 58 * XROW
TL = 112
NT = 28
N_CORES = 8
B_LOC = B // N_CORES
TX = 28                # winograd column tiles (pairs of output columns)
NG = 392               # matmul N: 14 output rows x 28 tiles
NGRP = 4               # row groups of 14
VCH = 16 * TX          # V chunk per ct: 16 rows x 28 tiles


def _tid1(j, kh, ct, oc):
    return ((j * 3 + kh) * 2 + ct) * 2 + oc


def _install_ntff_shim():
    name = "antenv.axon_hooks"
    if name in sys.modules:
        return
    try:
        import antenv  # noqa: F401

        m = types.ModuleType(name)
        m._hook = None
        m.set_axon_ntff_profile_hook = lambda h: setattr(m, "_hook", h)
        m.get_axon_ntff_profile_hook = lambda: m._hook
        sys.modules[name] = m
        setattr(sys.modules["antenv"], "axon_hooks", m)
        from trn_agent_boot.trn_boot import _ntff_profile_via_ctypes

        hook = _ntff_profile_via_ctypes("/opt/axon/libaxon_pjrt.so")
        if hook is not None:
            m.set_axon_ntff_profile_hook(hook)
    except Exception:
        pass


def _build_kernel():
    import concourse.bass as bass
    import concourse.mybir as mybir
    from concourse import bacc, tile
    from concourse.masks import make_identity
    from contextlib import ExitStack

    F32 = mybir.dt.float32
    BF16 = mybir.dt.bfloat16
    I16 = mybir.dt.int16

    nc = bacc.Bacc("TRN2", target_bir_lowering=False, debug=False, num_devices=N_CORES)

    xbt = nc.dram_tensor("xbt", [B_LOC, HW, C], BF16, kind="ExternalInput")
    u1 = nc.dram_tensor("u1", [48, 128, 128], BF16, kind="ExternalInput")
    idxt = nc.dram_tensor("idxt", [128, NT * 512], I16, kind="ExternalInput")
    out = nc.dram_tensor("out", [B_LOC, C, HW], F32, kind="ExternalOutput")

    with tile.TileContext(nc) as tc, ExitStack() as ctx:
        const = ctx.enter_context(tc.tile_pool(name="const", bufs=1))
        usb = const.tile([128, 48 * 128], BF16)
        nc.sync.dma_start(
            out=usb[:, :],
            in_=bass.AP(u1, 0, [[128, 128], [128 * 128, 48], [1, 128]]),
        )
        idxsb = const.tile([128, NT * 512], I16)
        nc.sync.dma_start(out=idxsb[:, :], in_=idxt[:, :])
        ident = const.tile([128, 128], BF16)
        make_identity(nc, ident[:, :])

        xs_pool = ctx.enter_context(tc.tile_pool(name="xs", bufs=4))
        gin_pool = ctx.enter_context(tc.tile_pool(name="gin", bufs=8))
        sout_pool = ctx.enter_context(tc.tile_pool(name="sout", bufs=8))
        v_pool = ctx.enter_context(tc.tile_pool(name="vp", bufs=12))
        tt_pool = ctx.enter_context(tc.tile_pool(name="tt", bufs=4))
        y_pool = ctx.enter_context(tc.tile_pool(name="yb", bufs=4))
        tpsB_pool = ctx.enter_context(tc.tile_pool(name="tpsB", bufs=3, space="PSUM"))
        z_pool = ctx.enter_context(tc.tile_pool(name="zp", bufs=4, space="PSUM"))

        def grp_block(b, xs, g):
            # d_b view for row group g: [p, yy, ct, tx, 1] =
            #   xs[p, (g*14+yy)*116 + ct*58 + 2*tx + b_]
            def dview(b_, g):
                v = xs[:, :].rearrange("p (yy u) -> p yy u", u=XROW)
                v = v[:, g * 14 : g * 14 + 16, :]
                v = v.rearrange("p yy (ct u2) -> p yy ct u2", ct=2)
                v = v.rearrange("p yy ct (vv ww) -> p yy ct vv ww", ww=2)
                return v[:, :, :, b_ // 2 : b_ // 2 + TX, b_ % 2 : b_ % 2 + 1]

            if True:
                vt = []
                for j in range(4):
                    vj = v_pool.tile([128, 2 * VCH], BF16, name="vj")
                    vv = vj[:, :].rearrange(
                        "p (ct yy tx q) -> p yy ct tx q", ct=2, tx=TX, q=1
                    )
                    if j == 0:
                        nc.vector.tensor_sub(vv, dview(0, g), dview(2, g))
                    elif j == 1:
                        nc.vector.tensor_add(vv, dview(1, g), dview(2, g))
                    elif j == 2:
                        nc.vector.tensor_sub(vv, dview(2, g), dview(1, g))
                    else:
                        nc.vector.tensor_sub(vv, dview(1, g), dview(3, g))
                    vt.append(vj)

                for oc in range(2):
                    ybuf = y_pool.tile([128, 14 * 56], F32, name="ybuf")
                    yv = ybuf[:, :].rearrange("p (y c q) -> p y c q", y=14, q=2)
                    zt = {}
                    for stage, js in enumerate(((1, 2), (0, 3))):
                        for j in js:
                            z = z_pool.tile([128, NG], F32, name="z")
                            k = 0
                            for kh in range(3):
                                for ct in range(2):
                                    nc.tensor.matmul(
                                        z[:, :],
                                        lhsT=usb[
                                            :,
                                            _tid1(j, kh, ct, oc) * 128 : (
                                                _tid1(j, kh, ct, oc) + 1
                                            )
                                            * 128,
                                        ],
                                        rhs=vt[j][
                                            :,
                                            ct * VCH
                                            + kh * TX : ct * VCH
                                            + kh * TX
                                            + NG,
                                        ],
                                        start=(k == 0),
                                        stop=(k == 5),
                                    )
                                    k += 1
                            zt[j] = z
                        if stage == 0:
                            zc = tt_pool.tile([128, NG], F32, name="zc")
                            nc.scalar.copy(zc[:, :], zt[1][:, :])
                            t0 = tt_pool.tile([128, NG], F32, name="t0")
                            t1 = tt_pool.tile([128, NG], F32, name="t0")
                            nc.vector.tensor_add(t0[:, :], zc[:, :], zt[2][:, :])
                            nc.vector.tensor_sub(t1[:, :], zc[:, :], zt[2][:, :])
                    tshape = lambda ap: ap.rearrange(
                        "p (y c q) -> p y c q", y=14, c=28, q=1
                    )
                    nc.vector.tensor_add(
                        yv[:, :, :, 0:1], tshape(t0[:, :]), tshape(zt[0][:, :])
                    )
                    nc.vector.tensor_sub(
                        yv[:, :, :, 1:2], tshape(t1[:, :]), tshape(zt[3][:, :])
                    )
                    nc.sync.dma_start(
                        out=out[
                            b,
                            oc * 128 : (oc + 1) * 128,
                            g * 14 * 56 : (g + 1) * 14 * 56,
                        ],
                        in_=ybuf[:, :],
                    )

        from collections import deque
        from functools import partial

        pending = deque()

        def flush(k):
            for _ in range(min(k, len(pending))):
                pending.popleft()()

        for pair in range(B_LOC // 2):
            b0, b1 = 2 * pair, 2 * pair + 1
            xst = {}
            for b in (b0, b1):
                xs = xs_pool.tile([128, XS_TOT], BF16)
                xst[b] = xs
                xv = xs[:, :].rearrange("p (yy u) -> p yy u", u=XROW)
                nc.vector.memset(xs[:, 0:XROW], 0.0)
                nc.vector.memset(xs[:, 57 * XROW : XS_TOT], 0.0)
                for ct in range(2):
                    nc.vector.memset(xv[:, 1:57, ct * PADW : ct * PADW + 1], 0.0)
                    nc.vector.memset(
                        xv[:, 1:57, ct * PADW + 57 : ct * PADW + 58], 0.0
                    )

            for t in range(NT):
                gin = gin_pool.tile([128, 512], BF16, name="gin", tag="gin")
                for i, b in enumerate((b0, b1)):
                    nc.sync.dma_start(
                        out=gin[0:TL, i * 256 : (i + 1) * 256],
                        in_=xbt[b, t * TL : (t + 1) * TL, :],
                    )
                sout = sout_pool.tile([128, 512], BF16, name="sout", tag="sout")
                nc.gpsimd.local_scatter(
                    out_ap=sout[0:TL, :],
                    data_ap=gin[0:TL, :],
                    idxs_ap=idxsb[0:TL, t * 512 : (t + 1) * 512],
                    channels=TL,
                    num_elems=512,
                    num_idxs=512,
                )
                for i, b in enumerate((b0, b1)):
                    ps2 = tpsB_pool.tile([128, 2 * TL], BF16, name="ps2", tag="ps2")
                    for ct in range(2):
                        nc.tensor.transpose(
                            ps2[:, ct * TL : ct * TL + TL],
                            sout[0:TL, i * 256 + ct * 128 : i * 256 + ct * 128 + 128],
                            ident[0:TL, 0:TL],
                        )
                    for ct in range(2):
                        q = (2 * t + 1) * XROW + ct * PADW + 1
                        dst = xst[b][:, q : q + 2 * XROW].rearrange(
                            "p (r x) -> p r x", r=2
                        )[:, :, 0:56]
                        src = ps2[:, ct * TL : ct * TL + TL].rearrange(
                            "p (r x) -> p r x", r=2
                        )
                        if ct == 0:
                            nc.scalar.copy(dst, src)
                        else:
                            nc.vector.tensor_copy(dst, src)
                # queue this pair's row-group conv blocks as their rows land
                if t >= 7 and (t - 7) % 7 == 0 and (t - 7) // 7 < 3:
                    gg = (t - 7) // 7
                    pending.append(partial(grp_block, b0, xst[b0], gg))
                    pending.append(partial(grp_block, b1, xst[b1], gg))
                elif t == NT - 1:
                    pending.append(partial(grp_block, b0, xst[b0], 3))
                    pending.append(partial(grp_block, b1, xst[b1], 3))
                if t % 4 == 3:
                    flush(2 if (pair == 1 and t >= 19) else 1)
        flush(len(pending))

    nc.compile()
    return nc


def _host_prep(x, w, perm):
    import ml_dtypes

    # [B, HW, C] pixel-major bf16 (feeds the scatter without PE fwd transposes)
    xft = np.ascontiguousarray(
        x.reshape(B, C, HW).transpose(0, 2, 1)
    ).astype(ml_dtypes.bfloat16)

    # 1-D winograd weights: U1[j,kh][oc,ic] = sum_kw G[j,kw] w[oc,ic,kh,kw]
    G = np.array([[1, 0, 0], [0.5, 0.5, 0.5], [0.5, -0.5, 0.5], [0, 0, 1]], np.float32)
    U1 = np.einsum("jk,ochk->jhoc", G, w.astype(np.float32))  # [4,3,OC,C]
    u1t = np.empty((48, 128, 128), dtype=ml_dtypes.bfloat16)
    for j in range(4):
        for kh in range(3):
            for ct in range(2):
                for oc in range(2):
                    blk = U1[j, kh][oc * 128 : (oc + 1) * 128, ct * 128 : (ct + 1) * 128]
                    u1t[_tid1(j, kh, ct, oc)] = blk.T.astype(ml_dtypes.bfloat16)

    iperm = np.empty((HW, C), dtype=np.int16)
    np.put_along_axis(
        iperm, perm.astype(np.int64), np.arange(C, dtype=np.int16)[None, :], axis=1
    )
    idxt = np.zeros((128, NT * 512), dtype=np.int16)
    for t in range(NT):
        blk = iperm[t * TL : t * TL + TL, :]
        idxt[0:TL, t * 512 : t * 512 + 256] = blk
        idxt[0:TL, t * 512 + 256 : (t + 1) * 512] = blk + 256

    in_maps = []
    for cidx in range(N_CORES):
        in_maps.append(
            {
                "xbt": np.ascontiguousarray(xft[cidx * B_LOC : (cidx + 1) * B_LOC]),
                "u1": u1t,
                "idxt": idxt,
            }
        )
    return in_maps


def kernel(x, w, perm):
    global LAST_RESULT
    _install_ntff_shim()
    from concourse.bass_utils import run_bass_kernel_spmd

    x = np.asarray(x, dtype=np.float32)
    w = np.asarray(w, dtype=np.float32)
    perm = np.asarray(perm)

    if "nc" not in _STATE:
        _STATE["nc"] = _build_kernel()
    nc = _STATE["nc"]

    in_maps = _host_prep(x, w, perm)
    res = run_bass_kernel_spmd(nc, in_maps, core_ids=list(range(N_CORES)))
    LAST_RESULT = res
    out = np.concatenate(
        [r["out"].reshape(B_LOC, C, H, W) for r in res.results], axis=0
    )
    return out.astype(np.float32)
